# revision 1
# baseline (speedup 1.0000x reference)
"""EntropyGraph Trainium2 kernel.

Computes, per batch b (one NeuronCore per batch):
  qt = heads(queries @ Wq_w.T + Wq_b), kt = heads(keys @ Wk_w.T + Wk_b)
  out[b,h,i,j] = -0.5 * sum_m log(1 - corr_m(i,j)^2 + eps)
where corr_m is the lag-m cross-correlation between query series i and key
series j within each head.

Strategy: correlation = alpha_i * G[i,j] where G = PE Gram matmul of
(raw q rows + mean-augmentation row) against (beta-scaled k rows +
-s1y-augmentation row); one-sided centering makes the mean correction exact.
alpha folds into the per-partition scale of the PSUM-evacuation passes.
Elementwise stage: ACT Square -> f16 DVE chain -> single ACT Ln -> DVE
final scale. Plain copies ride HWDGE (nc.sync); only broadcasts use SWDGE.
"""

import sys

import numpy as np

sys.path.insert(0, "/opt/trn_rl_repo")

import concourse.bacc as bacc
import concourse.tile as tile
from concourse import mybir
from concourse.bass_utils import run_bass_kernel_spmd

F32 = mybir.dt.float32
F32R = mybir.dt.float32r
F16 = mybir.dt.float16
OP = mybir.AluOpType
AF = mybir.ActivationFunctionType

B, N, DF = 8, 1024, 128
H, DK = 8, 64
EPS = 1e-6
C = 1.0 + EPS
NCHUNK = 4  # o-chunks of 128 in the 512-wide projection


def _build_nc():
    nc = bacc.Bacc("TRN2", target_bir_lowering=False)

    qT = nc.dram_tensor("qT", [DF, N], F32, kind="ExternalInput")
    kT = nc.dram_tensor("kT", [DF, N], F32, kind="ExternalInput")
    wqT = nc.dram_tensor("wqT", [DF, 512], F32, kind="ExternalInput")
    wkT = nc.dram_tensor("wkT", [DF, 512], F32, kind="ExternalInput")
    bq = nc.dram_tensor("bq", [128, 4], F32, kind="ExternalInput")
    bk = nc.dram_tensor("bk", [128, 4], F32, kind="ExternalInput")
    xmask = nc.dram_tensor("xmask", [128, 64], F32, kind="ExternalInput")
    ymask = nc.dram_tensor("ymask", [128, 64], F32, kind="ExternalInput")
    invn = nc.dram_tensor("invn", [16, 1], F32, kind="ExternalInput")
    ident = nc.dram_tensor("ident", [128, 128], F32, kind="ExternalInput")
    out = nc.dram_tensor("out", [H, N, N], F16, kind="ExternalOutput")
    # DRAM bounce buffer for beta: SBUF sources cannot use partition-step-0
    # (broadcast) APs, DRAM sources can.
    betad = nc.dram_tensor("betad", [16, N], F32R, kind="Internal")

    with tile.TileContext(nc) as tc:
        with tc.tile_pool(name="const", bufs=1) as const, \
             tc.tile_pool(name="proj", bufs=1) as projp, \
             tc.tile_pool(name="stats", bufs=1) as statp, \
             tc.tile_pool(name="scratch", bufs=2) as scratch:

            # ---- Stage A: load inputs -------------------------------------
            qT_s = const.tile([DF, N], F32)
            kT_s = const.tile([DF, N], F32)
            wqT_s = const.tile([DF, 512], F32)
            wkT_s = const.tile([DF, 512], F32)
            bq_s = const.tile([128, 4], F32)
            bk_s = const.tile([128, 4], F32)
            xm_s = const.tile([128, 64], F32)
            ym_s = const.tile([128, 64], F32)
            invn_s = const.tile([16, 1], F32)
            id_s = const.tile([128, 128], F32)
            for dst, src in ((qT_s, qT), (kT_s, kT), (wqT_s, wqT),
                             (wkT_s, wkT), (bq_s, bq), (bk_s, bk),
                             (xm_s, xmask), (ym_s, ymask), (invn_s, invn),
                             (id_s, ident)):
                nc.sync.dma_start(out=dst, in_=src[:, :])

            # masks rounded to f32r for the (f32r) stats matmuls
            xm_r = const.tile([128, 64], F32R)
            ym_r = const.tile([128, 64], F32R)
            nc.scalar.copy(xm_r, xm_s)
            nc.scalar.copy(ym_r, ym_s)

            # ---- Stage B: projections (transposed layout) -----------------
            # projT[o, n] = W[o, :] @ inT[:, n] + b[o]; fp32 matmul (inputs
            # come straight from DMA so they cannot be f32r), ACT evac
            # rounds to f32r and adds the per-partition bias.
            qproj = []
            kproj = []
            with tc.tile_pool(name="pps", bufs=2, space="PSUM") as pps:
                for (src_s, w_s, b_s, dst_list) in (
                        (kT_s, wkT_s, bk_s, kproj),
                        (qT_s, wqT_s, bq_s, qproj)):
                    for c in range(NCHUNK):
                        psb = pps.tile([128, N], F32)
                        for jh in range(2):
                            nc.tensor.matmul(
                                psb[:, jh * 512:(jh + 1) * 512],
                                lhsT=w_s[:, c * 128:(c + 1) * 128],
                                rhs=src_s[:, jh * 512:(jh + 1) * 512],
                                start=True, stop=True)
                        pt = projp.tile([128, N], F32R, tag=f"proj{len(dst_list)}_{c}_{w_s is wkT_s}")
                        nc.scalar.activation(out=pt, in_=psb, func=AF.Identity,
                                             bias=b_s[:, c:c + 1], scale=1.0)
                        dst_list.append(pt)

            # ---- Stage C: raw moments via mask matmuls --------------------
            # s1[r, n] = sum_d proj[d, n] over the (head, m) range; r = 2h+m-1
            # s2 likewise over squared projections.
            sq_list = {}
            with tc.tile_pool(name="sqp", bufs=2) as sqp:
                for name, plist in (("k", kproj), ("q", qproj)):
                    for c in range(NCHUNK):
                        sq = sqp.tile([128, N], F32R, tag=f"sq{name}{c}")
                        nc.scalar.activation(out=sq, in_=plist[c],
                                             func=AF.Square, bias=0.0, scale=1.0)
                        sq_list[(name, c)] = sq

                stats_sb = {}
                with tc.tile_pool(name="sps", bufs=1, space="PSUM") as sps:
                    for name, plist, mask in (("k", kproj, ym_r),
                                              ("q", qproj, xm_r)):
                        ps1 = sps.tile([16, N], F32, tag=f"ps1{name}")
                        ps2 = sps.tile([16, N], F32, tag=f"ps2{name}")
                        for c in range(NCHUNK):
                            for jh in range(2):
                                sl = slice(jh * 512, (jh + 1) * 512)
                                nc.tensor.matmul(
                                    ps1[:, sl],
                                    lhsT=mask[:, 16 * c:16 * c + 16],
                                    rhs=plist[c][:, sl],
                                    start=(c == 0), stop=(c == NCHUNK - 1))
                                nc.tensor.matmul(
                                    ps2[:, sl],
                                    lhsT=mask[:, 16 * c:16 * c + 16],
                                    rhs=sq_list[(name, c)][:, sl],
                                    start=(c == 0), stop=(c == NCHUNK - 1))
                        s1 = statp.tile([16, N], F32, tag=f"s1{name}")
                        s2 = statp.tile([16, N], F32, tag=f"s2{name}")
                        nc.scalar.copy(s1, ps1)
                        nc.scalar.copy(s2, ps2)
                        stats_sb[name] = (s1, s2)

            # ---- Stage D: stats math --------------------------------------
            s1q, s2q = stats_sb["q"]
            s1k, s2k = stats_sb["k"]
            invn_ap = invn_s[:, 0:1]

            # k-side: ns1y = -s1y (f32r, aug rows); beta = 1/sqrt(ssy) (f32r)
            ns1y = statp.tile([16, N], F32R)
            nc.vector.tensor_scalar(out=ns1y, in0=s1k, scalar1=-1.0,
                                    scalar2=None, op0=OP.mult)
            tk = scratch.tile([16, N], F32, tag="tq")
            nc.vector.tensor_mul(tk, s1k, s1k)
            nssy = scratch.tile([16, N], F32, tag="nssx")
            nc.vector.scalar_tensor_tensor(out=nssy, in0=tk, scalar=invn_ap,
                                           in1=s2k, op0=OP.mult, op1=OP.subtract)
            ryn = scratch.tile([16, N], F32, tag="rxn")
            nc.vector.reciprocal(ryn, nssy)
            beta16 = statp.tile([16, N], F32R)
            nc.scalar.activation(out=beta16, in_=ryn, func=AF.Sqrt,
                                 bias=0.0, scale=-1.0)
            nc.sync.dma_start(out=betad[:, :], in_=beta16)

            # mx = s1x / n_eff  (f32r: feeds augmentation rows via DMA)
            mx = statp.tile([16, N], F32R)
            nc.vector.tensor_scalar(out=mx, in0=s1q, scalar1=invn_ap,
                                    scalar2=None, op0=OP.mult)
            # -ssx = s1x^2/n - s2x ; alpha = sqrt(-1/(-ssx)) = 1/sqrt(ssx)
            tq = scratch.tile([16, N], F32, tag="tq")
            nc.vector.tensor_mul(tq, s1q, s1q)
            nssx = scratch.tile([16, N], F32, tag="nssx")
            nc.vector.scalar_tensor_tensor(out=nssx, in0=tq, scalar=invn_ap,
                                           in1=s2q, op0=OP.mult, op1=OP.subtract)
            rxn = scratch.tile([16, N], F32, tag="rxn")
            nc.vector.reciprocal(rxn, nssx)
            alpha16 = statp.tile([16, N], F32)
            nc.scalar.activation(out=alpha16, in_=rxn, func=AF.Sqrt,
                                 bias=0.0, scale=-1.0)

            # alpha transposed to [128, 8*16]: col ic*16 + (2h + m - 1)
            alphaT = statp.tile([128, 128], F32)
            with tc.tile_pool(name="tps", bufs=1, space="PSUM") as tps:
                pst = tps.tile([128, 128], F32)
                for ic in range(8):
                    nc.tensor.transpose(pst[:, ic * 16:(ic + 1) * 16],
                                        in_=alpha16[:, ic * 128:(ic + 1) * 128],
                                        identity=id_s[0:16, 0:16])
                nc.scalar.copy(alphaT, pst)

            # m1 augmentation: overwrite q_projT row rb+63 (unused d=63) with mx1
            for h in range(H):
                ch, rb = h // 2, (h % 2) * 64
                nc.sync.dma_start(out=qproj[ch][rb + 63:rb + 64, :],
                                    in_=mx[2 * h:2 * h + 1, :])

            # ---- Stage E: per-head Grams + elementwise --------------------
            with tc.tile_pool(name="head", bufs=2) as headp, \
                 tc.tile_pool(name="nsq", bufs=4) as nsq, \
                 tc.tile_pool(name="gps", bufs=2, space="PSUM") as gps:
                for h in range(H):
                    ch, rb = h // 2, (h % 2) * 64
                    yo1, yo2 = rb, 64 - rb
                    r1, r2 = 2 * h, 2 * h + 1

                    # Y raw: m1 block rows yo1..yo1+63 (k d=1..63 + aug),
                    #        m2 block rows yo2..yo2+62 (k d=2..63 + aug)
                    yraw = headp.tile([128, N], F32R, tag="yraw")
                    nc.sync.dma_start(out=yraw[yo1:yo1 + 63, :],
                                        in_=kproj[ch][rb + 1:rb + 64, :])
                    nc.sync.dma_start(out=yraw[yo1 + 63:yo1 + 64, :],
                                        in_=ns1y[r1:r1 + 1, :])
                    nc.sync.dma_start(out=yraw[yo2:yo2 + 62, :],
                                        in_=kproj[ch][rb + 2:rb + 64, :])
                    nc.sync.dma_start(out=yraw[yo2 + 62:yo2 + 63, :],
                                        in_=ns1y[r2:r2 + 1, :])
                    hole = yo2 + 63  # the single uncovered row
                    nc.sync.dma_start(out=yraw[hole:hole + 1, :],
                                        in_=ns1y[r1:r1 + 1, :])

                    bb = headp.tile([128, N], F32R, tag="bb")
                    nc.gpsimd.dma_start(
                        out=bb[yo1:yo1 + 64, :],
                        in_=betad[r1:r1 + 1, :].to_broadcast((64, N)))
                    nc.gpsimd.dma_start(
                        out=bb[yo2:yo2 + 64, :],
                        in_=betad[r2:r2 + 1, :].to_broadcast((64, N)))

                    yt = headp.tile([128, N], F32R, tag="yt")
                    nc.vector.tensor_mul(yt, yraw, bb)

                    # X2: m2 lhsT block at rows yo2..yo2+62 (q d=0..61 + mx2)
                    x2 = headp.tile([128, N], F32R, tag="x2")
                    nc.sync.dma_start(out=x2[yo2:yo2 + 62, :],
                                        in_=qproj[ch][rb:rb + 62, :])
                    nc.sync.dma_start(out=x2[yo2 + 62:yo2 + 63, :],
                                        in_=mx[r2:r2 + 1, :])

                    for ic in range(8):
                        isl = slice(ic * 128, (ic + 1) * 128)
                        psg1 = gps.tile([128, N], F32, tag="psg1")
                        psg2 = gps.tile([128, N], F32, tag="psg2")
                        for jh in range(2):
                            jsl = slice(jh * 512, (jh + 1) * 512)
                            nc.tensor.matmul(psg1[:, jsl],
                                             lhsT=qproj[ch][rb:rb + 64, isl],
                                             rhs=yt[yo1:yo1 + 64, jsl],
                                             start=True, stop=True)
                            nc.tensor.matmul(psg2[:, jsl],
                                             lhsT=x2[yo2:yo2 + 63, isl],
                                             rhs=yt[yo2:yo2 + 63, jsl],
                                             start=True, stop=True)
                        a1 = alphaT[:, ic * 16 + r1:ic * 16 + r1 + 1]
                        a2 = alphaT[:, ic * 16 + r2:ic * 16 + r2 + 1]
                        sg1 = nsq.tile([128, N], F16, tag="sg1")
                        nc.scalar.activation(out=sg1, in_=psg1, func=AF.Square,
                                             bias=0.0, scale=a1)
                        h1 = nsq.tile([128, N], F16, tag="h1")
                        nc.vector.tensor_scalar(out=h1, in0=sg1, scalar1=-1.0,
                                                scalar2=C, op0=OP.mult, op1=OP.add)
                        u = nsq.tile([128, N], F16, tag="u")
                        if False:  # ACT-both evac measured slower (518us vs 185us body)
                            # ACT-heavy variant: evacuate G2 as a second Square
                            sg2 = nsq.tile([128, N], F16, tag="sg2")
                            nc.scalar.activation(out=sg2, in_=psg2,
                                                 func=AF.Square, bias=0.0,
                                                 scale=a2)
                            h2 = nsq.tile([128, N], F16, tag="h2")
                            nc.vector.tensor_scalar(out=h2, in0=sg2, scalar1=-1.0,
                                                    scalar2=C, op0=OP.mult,
                                                    op1=OP.add)
                            nc.vector.tensor_mul(u, h1, h2)
                        else:
                            # DVE-heavy variant: evacuate G2 on the vector engine
                            e2 = nsq.tile([128, N], F16, tag="e2")
                            nc.vector.tensor_scalar(out=e2, in0=psg2, scalar1=a2,
                                                    scalar2=None, op0=OP.mult)
                            n2 = nsq.tile([128, N], F16, tag="n2")
                            nc.vector.scalar_tensor_tensor(out=n2, in0=e2,
                                                           scalar=-1.0, in1=e2,
                                                           op0=OP.mult,
                                                           op1=OP.mult)
                            nc.vector.scalar_tensor_tensor(out=u, in0=n2,
                                                           scalar=C, in1=h1,
                                                           op0=OP.add,
                                                           op1=OP.mult)
                        lt = nsq.tile([128, N], F16, tag="lt")
                        nc.scalar.activation(out=lt, in_=u, func=AF.Ln,
                                             bias=0.0, scale=1.0)
                        o = nsq.tile([128, N], F16, tag="o")
                        nc.vector.tensor_scalar(out=o, in0=lt, scalar1=-0.5,
                                                scalar2=None, op0=OP.mult)
                        nc.sync.dma_start(out=out[h, isl, :], in_=o)
    nc.compile()
    return nc


_NC = None


def _get_nc():
    global _NC
    if _NC is None:
        _NC = _build_nc()
    return _NC


def _host_inputs(queries, keys, Wq_w, Wq_b, Wk_w, Wk_b):
    qT = np.ascontiguousarray(queries.transpose(0, 2, 1), dtype=np.float32)
    kT = np.ascontiguousarray(keys.transpose(0, 2, 1), dtype=np.float32)
    wqT = np.ascontiguousarray(Wq_w.T, dtype=np.float32)
    wkT = np.ascontiguousarray(Wk_w.T, dtype=np.float32)
    bq = np.ascontiguousarray(Wq_b.reshape(4, 128).T, dtype=np.float32)
    bk = np.ascontiguousarray(Wk_b.reshape(4, 128).T, dtype=np.float32)

    xmask = np.zeros((128, 64), dtype=np.float32)
    ymask = np.zeros((128, 64), dtype=np.float32)
    for c in range(4):
        for hp in range(2):
            for m in (1, 2):
                j = 4 * c + 2 * hp + (m - 1)      # output partition row r
                col = 16 * c + j                   # column within this chunk's mask
                rows = np.arange(hp * 64, hp * 64 + 64 - m)
                xmask[rows, col] = 1.0
                yrows = np.arange(hp * 64 + m, hp * 64 + 64)
                ymask[yrows, col] = 1.0

    invn = np.array([[1.0 / (64 - ((r % 2) + 1))] for r in range(16)],
                    dtype=np.float32)
    ident = np.eye(128, dtype=np.float32)

    shared = dict(wqT=wqT, wkT=wkT, bq=bq, bk=bk, xmask=xmask, ymask=ymask,
                  invn=invn, ident=ident)
    in_maps = []
    for b in range(B):
        m = dict(shared)
        m["qT"] = np.ascontiguousarray(qT[b])
        m["kT"] = np.ascontiguousarray(kT[b])
        in_maps.append(m)
    return in_maps


def kernel(queries, keys, Wq_w, Wq_b, Wk_w, Wk_b):
    nc = _get_nc()
    in_maps = _host_inputs(np.asarray(queries), np.asarray(keys),
                           np.asarray(Wq_w), np.asarray(Wq_b),
                           np.asarray(Wk_w), np.asarray(Wk_b))
    res = run_bass_kernel_spmd(nc, in_maps, core_ids=list(range(B)))
    out = np.stack([res.results[b]["out"].astype(np.float32) for b in range(B)],
                   axis=0)
    return out



# revision 3
# speedup vs baseline: 9811.6008x; 9811.6008x over previous
"""EntropyGraph Trainium2 kernel.

Computes, per batch b (one NeuronCore per batch):
  qt = heads(queries @ Wq_w.T + Wq_b), kt = heads(keys @ Wk_w.T + Wk_b)
  out[b,h,i,j] = -0.5 * sum_m log(1 - corr_m(i,j)^2 + eps)
where corr_m is the lag-m cross-correlation between query series i and key
series j within each head.

Strategy: correlation = alpha_i * G[i,j] where G = PE Gram matmul of
(raw q rows + mean-augmentation row) against (beta-scaled k rows +
-s1y-augmentation row); one-sided centering makes the mean correction exact.
alpha folds into the per-partition scale of the PSUM-evacuation passes.
Elementwise stage: ACT Square -> f16 DVE chain -> single ACT Ln -> DVE
final scale. Plain copies ride HWDGE (nc.sync); only broadcasts use SWDGE.
"""

import sys

import numpy as np

sys.path.insert(0, "/opt/trn_rl_repo")

import concourse.bacc as bacc
import concourse.tile as tile
from concourse import mybir
from concourse.bass_utils import run_bass_kernel_spmd

F32 = mybir.dt.float32
F32R = mybir.dt.float32r
F16 = mybir.dt.float16
OP = mybir.AluOpType
AF = mybir.ActivationFunctionType

B, N, DF = 8, 1024, 128
H, DK = 8, 64
EPS = 1e-6
C = 1.0 + EPS
NCHUNK = 4  # o-chunks of 128 in the 512-wide projection


def _emit_body(nc, tc, t):
    qT, kT, wqT, wkT, bq, bk, xmask, ymask, invn, ident, out, betad = t
    with tc.tile_pool(name="const", bufs=1) as const, \
         tc.tile_pool(name="proj", bufs=1) as projp, \
         tc.tile_pool(name="stats", bufs=1) as statp, \
         tc.tile_pool(name="scratch", bufs=2) as scratch:

        # ---- Stage A: load inputs -------------------------------------
        qT_s = const.tile([DF, N], F32)
        kT_s = const.tile([DF, N], F32)
        wqT_s = const.tile([DF, 512], F32)
        wkT_s = const.tile([DF, 512], F32)
        bq_s = const.tile([128, 4], F32)
        bk_s = const.tile([128, 4], F32)
        xm_s = const.tile([128, 64], F32)
        ym_s = const.tile([128, 64], F32)
        invn_s = const.tile([16, 1], F32)
        id_s = const.tile([128, 128], F32)
        for dst, src in ((qT_s, qT), (kT_s, kT), (wqT_s, wqT),
                         (wkT_s, wkT), (bq_s, bq), (bk_s, bk),
                         (xm_s, xmask), (ym_s, ymask), (invn_s, invn),
                         (id_s, ident)):
            nc.sync.dma_start(out=dst, in_=src[:, :])

        # masks rounded to f32r for the (f32r) stats matmuls
        xm_r = const.tile([128, 64], F32R)
        ym_r = const.tile([128, 64], F32R)
        nc.scalar.copy(xm_r, xm_s)
        nc.scalar.copy(ym_r, ym_s)

        # ---- Stage B: projections (transposed layout) -----------------
        # projT[o, n] = W[o, :] @ inT[:, n] + b[o]; fp32 matmul (inputs
        # come straight from DMA so they cannot be f32r), ACT evac
        # rounds to f32r and adds the per-partition bias.
        qproj = []
        kproj = []
        with tc.tile_pool(name="pps", bufs=2, space="PSUM") as pps:
            for (src_s, w_s, b_s, dst_list) in (
                    (kT_s, wkT_s, bk_s, kproj),
                    (qT_s, wqT_s, bq_s, qproj)):
                for c in range(NCHUNK):
                    psb = pps.tile([128, N], F32)
                    for jh in range(2):
                        nc.tensor.matmul(
                            psb[:, jh * 512:(jh + 1) * 512],
                            lhsT=w_s[:, c * 128:(c + 1) * 128],
                            rhs=src_s[:, jh * 512:(jh + 1) * 512],
                            start=True, stop=True)
                    pt = projp.tile([128, N], F32R, tag=f"proj{len(dst_list)}_{c}_{w_s is wkT_s}")
                    nc.scalar.activation(out=pt, in_=psb, func=AF.Identity,
                                         bias=b_s[:, c:c + 1], scale=1.0)
                    dst_list.append(pt)

        # ---- Stage C: raw moments via mask matmuls --------------------
        # s1[r, n] = sum_d proj[d, n] over the (head, m) range; r = 2h+m-1
        # s2 likewise over squared projections.
        sq_list = {}
        with tc.tile_pool(name="sqp", bufs=2) as sqp:
            for name, plist in (("k", kproj), ("q", qproj)):
                for c in range(NCHUNK):
                    sq = sqp.tile([128, N], F32R, tag=f"sq{name}{c}")
                    nc.scalar.activation(out=sq, in_=plist[c],
                                         func=AF.Square, bias=0.0, scale=1.0)
                    sq_list[(name, c)] = sq

            stats_sb = {}
            with tc.tile_pool(name="sps", bufs=1, space="PSUM") as sps:
                for name, plist, mask in (("k", kproj, ym_r),
                                          ("q", qproj, xm_r)):
                    ps1 = sps.tile([16, N], F32, tag=f"ps1{name}")
                    ps2 = sps.tile([16, N], F32, tag=f"ps2{name}")
                    for c in range(NCHUNK):
                        for jh in range(2):
                            sl = slice(jh * 512, (jh + 1) * 512)
                            nc.tensor.matmul(
                                ps1[:, sl],
                                lhsT=mask[:, 16 * c:16 * c + 16],
                                rhs=plist[c][:, sl],
                                start=(c == 0), stop=(c == NCHUNK - 1))
                            nc.tensor.matmul(
                                ps2[:, sl],
                                lhsT=mask[:, 16 * c:16 * c + 16],
                                rhs=sq_list[(name, c)][:, sl],
                                start=(c == 0), stop=(c == NCHUNK - 1))
                    s1 = statp.tile([16, N], F32, tag=f"s1{name}")
                    s2 = statp.tile([16, N], F32, tag=f"s2{name}")
                    nc.scalar.copy(s1, ps1)
                    nc.scalar.copy(s2, ps2)
                    stats_sb[name] = (s1, s2)

        # ---- Stage D: stats math --------------------------------------
        s1q, s2q = stats_sb["q"]
        s1k, s2k = stats_sb["k"]
        invn_ap = invn_s[:, 0:1]

        # k-side: ns1y = -s1y (f32r, aug rows); beta = 1/sqrt(ssy) (f32r)
        ns1y = statp.tile([16, N], F32R)
        nc.vector.tensor_scalar(out=ns1y, in0=s1k, scalar1=-1.0,
                                scalar2=None, op0=OP.mult)
        tk = scratch.tile([16, N], F32, tag="tq")
        nc.vector.tensor_mul(tk, s1k, s1k)
        nssy = scratch.tile([16, N], F32, tag="nssx")
        nc.vector.scalar_tensor_tensor(out=nssy, in0=tk, scalar=invn_ap,
                                       in1=s2k, op0=OP.mult, op1=OP.subtract)
        ryn = scratch.tile([16, N], F32, tag="rxn")
        nc.vector.reciprocal(ryn, nssy)
        beta16 = statp.tile([16, N], F32R)
        nc.scalar.activation(out=beta16, in_=ryn, func=AF.Sqrt,
                             bias=0.0, scale=-1.0)
        nc.sync.dma_start(out=betad[:, :], in_=beta16)

        # mx = s1x / n_eff  (f32r: feeds augmentation rows via DMA)
        mx = statp.tile([16, N], F32R)
        nc.vector.tensor_scalar(out=mx, in0=s1q, scalar1=invn_ap,
                                scalar2=None, op0=OP.mult)
        # -ssx = s1x^2/n - s2x ; alpha = sqrt(-1/(-ssx)) = 1/sqrt(ssx)
        tq = scratch.tile([16, N], F32, tag="tq")
        nc.vector.tensor_mul(tq, s1q, s1q)
        nssx = scratch.tile([16, N], F32, tag="nssx")
        nc.vector.scalar_tensor_tensor(out=nssx, in0=tq, scalar=invn_ap,
                                       in1=s2q, op0=OP.mult, op1=OP.subtract)
        rxn = scratch.tile([16, N], F32, tag="rxn")
        nc.vector.reciprocal(rxn, nssx)
        alpha16 = statp.tile([16, N], F32)
        nc.scalar.activation(out=alpha16, in_=rxn, func=AF.Sqrt,
                             bias=0.0, scale=-1.0)

        # alpha transposed to [128, 8*16]: col ic*16 + (2h + m - 1)
        alphaT = statp.tile([128, 128], F32)
        with tc.tile_pool(name="tps", bufs=1, space="PSUM") as tps:
            pst = tps.tile([128, 128], F32)
            for ic in range(8):
                nc.tensor.transpose(pst[:, ic * 16:(ic + 1) * 16],
                                    in_=alpha16[:, ic * 128:(ic + 1) * 128],
                                    identity=id_s[0:16, 0:16])
            nc.scalar.copy(alphaT, pst)

        # m1 augmentation: overwrite q_projT row rb+63 (unused d=63) with mx1
        for h in range(H):
            ch, rb = h // 2, (h % 2) * 64
            nc.sync.dma_start(out=qproj[ch][rb + 63:rb + 64, :],
                                in_=mx[2 * h:2 * h + 1, :])

        # ---- Stage E: per-head Grams + elementwise --------------------
        with tc.tile_pool(name="head", bufs=2) as headp, \
             tc.tile_pool(name="nsq", bufs=4) as nsq, \
             tc.tile_pool(name="gps", bufs=2, space="PSUM") as gps:
            for h in range(H):
                ch, rb = h // 2, (h % 2) * 64
                yo1, yo2 = rb, 64 - rb
                r1, r2 = 2 * h, 2 * h + 1

                # Y raw: m1 block rows yo1..yo1+63 (k d=1..63 + aug),
                #        m2 block rows yo2..yo2+62 (k d=2..63 + aug)
                yraw = headp.tile([128, N], F32R, tag="yraw")
                nc.sync.dma_start(out=yraw[yo1:yo1 + 63, :],
                                    in_=kproj[ch][rb + 1:rb + 64, :])
                nc.sync.dma_start(out=yraw[yo1 + 63:yo1 + 64, :],
                                    in_=ns1y[r1:r1 + 1, :])
                nc.sync.dma_start(out=yraw[yo2:yo2 + 62, :],
                                    in_=kproj[ch][rb + 2:rb + 64, :])
                nc.sync.dma_start(out=yraw[yo2 + 62:yo2 + 63, :],
                                    in_=ns1y[r2:r2 + 1, :])
                hole = yo2 + 63  # the single uncovered row
                nc.sync.dma_start(out=yraw[hole:hole + 1, :],
                                    in_=ns1y[r1:r1 + 1, :])

                bb = headp.tile([128, N], F32R, tag="bb")
                nc.gpsimd.dma_start(
                    out=bb[yo1:yo1 + 64, :],
                    in_=betad[r1:r1 + 1, :].to_broadcast((64, N)))
                nc.gpsimd.dma_start(
                    out=bb[yo2:yo2 + 64, :],
                    in_=betad[r2:r2 + 1, :].to_broadcast((64, N)))

                yt = headp.tile([128, N], F32R, tag="yt")
                nc.vector.tensor_mul(yt, yraw, bb)

                # X2: m2 lhsT block at rows yo2..yo2+62 (q d=0..61 + mx2)
                x2 = headp.tile([128, N], F32R, tag="x2")
                nc.sync.dma_start(out=x2[yo2:yo2 + 62, :],
                                    in_=qproj[ch][rb:rb + 62, :])
                nc.sync.dma_start(out=x2[yo2 + 62:yo2 + 63, :],
                                    in_=mx[r2:r2 + 1, :])

                for ic in range(8):
                    isl = slice(ic * 128, (ic + 1) * 128)
                    psg1 = gps.tile([128, N], F32, tag="psg1")
                    psg2 = gps.tile([128, N], F32, tag="psg2")
                    for jh in range(2):
                        jsl = slice(jh * 512, (jh + 1) * 512)
                        nc.tensor.matmul(psg1[:, jsl],
                                         lhsT=qproj[ch][rb:rb + 64, isl],
                                         rhs=yt[yo1:yo1 + 64, jsl],
                                         start=True, stop=True)
                        nc.tensor.matmul(psg2[:, jsl],
                                         lhsT=x2[yo2:yo2 + 63, isl],
                                         rhs=yt[yo2:yo2 + 63, jsl],
                                         start=True, stop=True)
                    a1 = alphaT[:, ic * 16 + r1:ic * 16 + r1 + 1]
                    a2 = alphaT[:, ic * 16 + r2:ic * 16 + r2 + 1]
                    sg1 = nsq.tile([128, N], F16, tag="sg1")
                    nc.scalar.activation(out=sg1, in_=psg1, func=AF.Square,
                                         bias=0.0, scale=a1)
                    h1 = nsq.tile([128, N], F16, tag="h1")
                    nc.vector.tensor_scalar(out=h1, in0=sg1, scalar1=-1.0,
                                            scalar2=C, op0=OP.mult, op1=OP.add)
                    u = nsq.tile([128, N], F16, tag="u")
                    # DVE-heavy variant: evacuate G2 on the vector engine
                    e2 = nsq.tile([128, N], F16, tag="e2")
                    nc.vector.tensor_scalar(out=e2, in0=psg2, scalar1=a2,
                                            scalar2=None, op0=OP.mult)
                    n2 = nsq.tile([128, N], F16, tag="n2")
                    nc.vector.scalar_tensor_tensor(out=n2, in0=e2,
                                                   scalar=-1.0, in1=e2,
                                                   op0=OP.mult,
                                                   op1=OP.mult)
                    nc.vector.scalar_tensor_tensor(out=u, in0=n2,
                                                   scalar=C, in1=h1,
                                                   op0=OP.add,
                                                   op1=OP.mult)
                    lt = nsq.tile([128, N], F16, tag="lt")
                    nc.scalar.activation(out=lt, in_=u, func=AF.Ln,
                                         bias=0.0, scale=1.0)
                    o = nsq.tile([128, N], F16, tag="o")
                    nc.vector.tensor_scalar(out=o, in0=lt, scalar1=-0.5,
                                            scalar2=None, op0=OP.mult)
                    nc.sync.dma_start(out=out[h, isl, :], in_=o)


def _build_nc(repeat=1):
    nc = bacc.Bacc("TRN2", target_bir_lowering=False)

    qT = nc.dram_tensor("qT", [DF, N], F32, kind="ExternalInput")
    kT = nc.dram_tensor("kT", [DF, N], F32, kind="ExternalInput")
    wqT = nc.dram_tensor("wqT", [DF, 512], F32, kind="ExternalInput")
    wkT = nc.dram_tensor("wkT", [DF, 512], F32, kind="ExternalInput")
    bq = nc.dram_tensor("bq", [128, 4], F32, kind="ExternalInput")
    bk = nc.dram_tensor("bk", [128, 4], F32, kind="ExternalInput")
    xmask = nc.dram_tensor("xmask", [128, 64], F32, kind="ExternalInput")
    ymask = nc.dram_tensor("ymask", [128, 64], F32, kind="ExternalInput")
    invn = nc.dram_tensor("invn", [16, 1], F32, kind="ExternalInput")
    ident = nc.dram_tensor("ident", [128, 128], F32, kind="ExternalInput")
    out = nc.dram_tensor("out", [H, N, N], F16, kind="ExternalOutput")
    # DRAM bounce buffer for beta: SBUF sources cannot use partition-step-0
    # (broadcast) APs, DRAM sources can.
    betad = nc.dram_tensor("betad", [16, N], F32R, kind="Internal")

    t = (qT, kT, wqT, wkT, bq, bk, xmask, ymask, invn, ident, out, betad)
    with tile.TileContext(nc) as tc:
        for _rep in range(repeat):
            _emit_body(nc, tc, t)
    nc.compile()
    return nc


_NC = None


def _get_nc():
    global _NC
    if _NC is None:
        _NC = _build_nc()
    return _NC


def _host_inputs(queries, keys, Wq_w, Wq_b, Wk_w, Wk_b):
    qT = np.ascontiguousarray(queries.transpose(0, 2, 1), dtype=np.float32)
    kT = np.ascontiguousarray(keys.transpose(0, 2, 1), dtype=np.float32)
    wqT = np.ascontiguousarray(Wq_w.T, dtype=np.float32)
    wkT = np.ascontiguousarray(Wk_w.T, dtype=np.float32)
    bq = np.ascontiguousarray(Wq_b.reshape(4, 128).T, dtype=np.float32)
    bk = np.ascontiguousarray(Wk_b.reshape(4, 128).T, dtype=np.float32)

    xmask = np.zeros((128, 64), dtype=np.float32)
    ymask = np.zeros((128, 64), dtype=np.float32)
    for c in range(4):
        for hp in range(2):
            for m in (1, 2):
                j = 4 * c + 2 * hp + (m - 1)      # output partition row r
                col = 16 * c + j                   # column within this chunk's mask
                rows = np.arange(hp * 64, hp * 64 + 64 - m)
                xmask[rows, col] = 1.0
                yrows = np.arange(hp * 64 + m, hp * 64 + 64)
                ymask[yrows, col] = 1.0

    invn = np.array([[1.0 / (64 - ((r % 2) + 1))] for r in range(16)],
                    dtype=np.float32)
    ident = np.eye(128, dtype=np.float32)

    shared = dict(wqT=wqT, wkT=wkT, bq=bq, bk=bk, xmask=xmask, ymask=ymask,
                  invn=invn, ident=ident)
    in_maps = []
    for b in range(B):
        m = dict(shared)
        m["qT"] = np.ascontiguousarray(qT[b])
        m["kT"] = np.ascontiguousarray(kT[b])
        in_maps.append(m)
    return in_maps


def kernel(queries, keys, Wq_w, Wq_b, Wk_w, Wk_b):
    nc = _get_nc()
    in_maps = _host_inputs(np.asarray(queries), np.asarray(keys),
                           np.asarray(Wq_w), np.asarray(Wq_b),
                           np.asarray(Wk_w), np.asarray(Wk_b))
    res = run_bass_kernel_spmd(nc, in_maps, core_ids=list(range(B)))
    out = np.stack([res.results[b]["out"].astype(np.float32) for b in range(B)],
                   axis=0)
    return out


# revision 8
# speedup vs baseline: 20610.5805x; 2.1006x over previous
"""EntropyGraph Trainium2 kernel.

Computes, per batch b (one NeuronCore per batch):
  qt = heads(queries @ Wq_w.T + Wq_b), kt = heads(keys @ Wk_w.T + Wk_b)
  out[b,h,i,j] = -0.5 * sum_m log(1 - corr_m(i,j)^2 + eps)
where corr_m is the lag-m cross-correlation between query series i and key
series j within each head.

Strategy: correlation = alpha_i * G[i,j] where G = PE Gram matmul of
(raw q rows + mean-augmentation row) against (beta-scaled k rows +
-s1y-augmentation row); one-sided centering makes the mean correction exact.
alpha folds into the per-partition scale of the PSUM-evacuation passes.
Elementwise stage: ACT Square -> f16 DVE chain -> single ACT Ln -> DVE
final scale. Plain copies ride HWDGE (nc.sync); only broadcasts use SWDGE.
"""

import sys

import numpy as np

sys.path.insert(0, "/opt/trn_rl_repo")

import concourse.bacc as bacc
import concourse.tile as tile
from concourse import mybir
from concourse.bass_utils import run_bass_kernel_spmd

F32 = mybir.dt.float32
F32R = mybir.dt.float32r
F16 = mybir.dt.float16
OP = mybir.AluOpType
AF = mybir.ActivationFunctionType

B, N, DF = 8, 1024, 128
H, DK = 8, 64
EPS = 1e-6
C = 1.0 + EPS
NCHUNK = 4  # o-chunks of 128 in the 512-wide projection


def _emit_body(nc, tc, t):
    qT, kT, wqT, wkT, bq, bk, xmask, ymask, invn, ident, out, betad = t
    with tc.tile_pool(name="const", bufs=1) as const, \
         tc.tile_pool(name="proj", bufs=1) as projp, \
         tc.tile_pool(name="stats", bufs=1) as statp, \
         tc.tile_pool(name="scratch", bufs=2) as scratch:

        # ---- Stage A: load inputs -------------------------------------
        qT_s = const.tile([DF, N], F32)
        kT_s = const.tile([DF, N], F32)
        wqT_s = const.tile([DF, 512], F32)
        wkT_s = const.tile([DF, 512], F32)
        bq_s = const.tile([128, 4], F32)
        bk_s = const.tile([128, 4], F32)
        xm_s = const.tile([128, 64], F32)
        ym_s = const.tile([128, 64], F32)
        invn_s = const.tile([16, 1], F32)
        id_s = const.tile([128, 128], F32)
        for dst, src in ((qT_s, qT), (kT_s, kT), (wqT_s, wqT),
                         (wkT_s, wkT), (bq_s, bq), (bk_s, bk),
                         (xm_s, xmask), (ym_s, ymask), (invn_s, invn),
                         (id_s, ident)):
            nc.sync.dma_start(out=dst, in_=src[:, :])

        # masks rounded to f32r for the (f32r) stats matmuls
        xm_r = const.tile([128, 64], F32R)
        ym_r = const.tile([128, 64], F32R)
        nc.scalar.copy(xm_r, xm_s)
        nc.scalar.copy(ym_r, ym_s)

        # ---- Stage B: projections (transposed layout) -----------------
        # projT[o, n] = W[o, :] @ inT[:, n] + b[o]; fp32 matmul (inputs
        # come straight from DMA so they cannot be f32r), ACT evac
        # rounds to f32r and adds the per-partition bias.
        qproj = []
        kproj = []
        with tc.tile_pool(name="pps", bufs=2, space="PSUM") as pps:
            for (src_s, w_s, b_s, dst_list) in (
                    (kT_s, wkT_s, bk_s, kproj),
                    (qT_s, wqT_s, bq_s, qproj)):
                for c in range(NCHUNK):
                    psb = pps.tile([128, N], F32)
                    for jh in range(2):
                        nc.tensor.matmul(
                            psb[:, jh * 512:(jh + 1) * 512],
                            lhsT=w_s[:, c * 128:(c + 1) * 128],
                            rhs=src_s[:, jh * 512:(jh + 1) * 512],
                            start=True, stop=True)
                    pt = projp.tile([128, N], F32R, tag=f"proj{len(dst_list)}_{c}_{w_s is wkT_s}")
                    nc.scalar.activation(out=pt, in_=psb, func=AF.Identity,
                                         bias=b_s[:, c:c + 1], scale=1.0)
                    dst_list.append(pt)

        # ---- Stage C: raw moments via mask matmuls --------------------
        # s1[r, n] = sum_d proj[d, n] over the (head, m) range; r = 2h+m-1
        # s2 likewise over squared projections.
        sq_list = {}
        with tc.tile_pool(name="sqp", bufs=2) as sqp:
            for name, plist in (("k", kproj), ("q", qproj)):
                for c in range(NCHUNK):
                    sq = sqp.tile([128, N], F32R, tag=f"sq{name}{c}")
                    nc.scalar.activation(out=sq, in_=plist[c],
                                         func=AF.Square, bias=0.0, scale=1.0)
                    sq_list[(name, c)] = sq

            stats_sb = {}
            with tc.tile_pool(name="sps", bufs=1, space="PSUM") as sps:
                for name, plist, mask in (("k", kproj, ym_r),
                                          ("q", qproj, xm_r)):
                    ps1 = sps.tile([16, N], F32, tag=f"ps1{name}")
                    ps2 = sps.tile([16, N], F32, tag=f"ps2{name}")
                    for c in range(NCHUNK):
                        for jh in range(2):
                            sl = slice(jh * 512, (jh + 1) * 512)
                            nc.tensor.matmul(
                                ps1[:, sl],
                                lhsT=mask[:, 16 * c:16 * c + 16],
                                rhs=plist[c][:, sl],
                                start=(c == 0), stop=(c == NCHUNK - 1))
                            nc.tensor.matmul(
                                ps2[:, sl],
                                lhsT=mask[:, 16 * c:16 * c + 16],
                                rhs=sq_list[(name, c)][:, sl],
                                start=(c == 0), stop=(c == NCHUNK - 1))
                    s1 = statp.tile([16, N], F32, tag=f"s1{name}")
                    s2 = statp.tile([16, N], F32, tag=f"s2{name}")
                    nc.scalar.copy(s1, ps1)
                    nc.scalar.copy(s2, ps2)
                    stats_sb[name] = (s1, s2)

        # ---- Stage D: stats math --------------------------------------
        s1q, s2q = stats_sb["q"]
        s1k, s2k = stats_sb["k"]
        invn_ap = invn_s[:, 0:1]

        # k-side: ns1y = -s1y (f32r, aug rows); beta = 1/sqrt(ssy) (f32r)
        ns1y = statp.tile([16, N], F32R)
        nc.vector.tensor_scalar(out=ns1y, in0=s1k, scalar1=-1.0,
                                scalar2=None, op0=OP.mult)
        tk = scratch.tile([16, N], F32, tag="tq")
        nc.vector.tensor_mul(tk, s1k, s1k)
        nssy = scratch.tile([16, N], F32, tag="nssx")
        nc.vector.scalar_tensor_tensor(out=nssy, in0=tk, scalar=invn_ap,
                                       in1=s2k, op0=OP.mult, op1=OP.subtract)
        ryn = scratch.tile([16, N], F32, tag="rxn")
        nc.vector.reciprocal(ryn, nssy)
        beta16 = statp.tile([16, N], F32R)
        nc.scalar.activation(out=beta16, in_=ryn, func=AF.Sqrt,
                             bias=0.0, scale=-1.0)
        nc.sync.dma_start(out=betad[:, :], in_=beta16)

        # mx = s1x / n_eff  (f32r: feeds augmentation rows via DMA)
        mx = statp.tile([16, N], F32R)
        nc.vector.tensor_scalar(out=mx, in0=s1q, scalar1=invn_ap,
                                scalar2=None, op0=OP.mult)
        # -ssx = s1x^2/n - s2x ; alpha = sqrt(-1/(-ssx)) = 1/sqrt(ssx)
        tq = scratch.tile([16, N], F32, tag="tq")
        nc.vector.tensor_mul(tq, s1q, s1q)
        nssx = scratch.tile([16, N], F32, tag="nssx")
        nc.vector.scalar_tensor_tensor(out=nssx, in0=tq, scalar=invn_ap,
                                       in1=s2q, op0=OP.mult, op1=OP.subtract)
        rxn = scratch.tile([16, N], F32, tag="rxn")
        nc.vector.reciprocal(rxn, nssx)
        alpha16 = statp.tile([16, N], F32)
        nc.scalar.activation(out=alpha16, in_=rxn, func=AF.Sqrt,
                             bias=0.0, scale=-1.0)

        # alpha transposed to [128, 8*16]: col ic*16 + (2h + m - 1)
        alphaT = statp.tile([128, 128], F32)
        with tc.tile_pool(name="tps", bufs=1, space="PSUM") as tps:
            pst = tps.tile([128, 128], F32)
            for ic in range(8):
                nc.tensor.transpose(pst[:, ic * 16:(ic + 1) * 16],
                                    in_=alpha16[:, ic * 128:(ic + 1) * 128],
                                    identity=id_s[0:16, 0:16])
            nc.scalar.copy(alphaT, pst)

        # m1 augmentation: overwrite q_projT row rb+63 (unused d=63) with mx1
        for h in range(H):
            ch, rb = h // 2, (h % 2) * 64
            nc.sync.dma_start(out=qproj[ch][rb + 63:rb + 64, :],
                                in_=mx[2 * h:2 * h + 1, :])

        # ---- Stage E: per-head Grams + elementwise (software-pipelined)
        # Flat iteration t = 8*h + ic.  Stages at skews:
        #   step t+0: PE Gram matmuls -> psg1/psg2
        #   step t+1: PSUM evacuation (ACT Square; type-B iters put the
        #             G2 evac on DVE instead to balance engine load)
        #   step t+2: DVE glue -> u = (C - s1)(C - s2)
        #   step t+3: ACT Ln
        #   step t+4: DVE -0.5 scale + DMA out
        # Emission is stage-sorted so neither engine's FIFO head blocks on
        # the other engine's in-flight work.
        T = H * 8
        with tc.tile_pool(name="head", bufs=2) as headp, \
             tc.tile_pool(name="sgp", bufs=4) as sgp, \
             tc.tile_pool(name="uup", bufs=4) as uup, \
             tc.tile_pool(name="ltp", bufs=4) as ltp, \
             tc.tile_pool(name="oop", bufs=3) as oop, \
             tc.tile_pool(name="gps", bufs=2, space="PSUM") as gps:

            heads = {}

            def prep_head(h):
                ch, rb = h // 2, (h % 2) * 64
                yo1, yo2 = rb, 64 - rb
                r1, r2 = 2 * h, 2 * h + 1
                # Y raw: m1 block rows yo1..yo1+63 (k d=1..63 + aug),
                #        m2 block rows yo2..yo2+62 (k d=2..63 + aug)
                yraw = headp.tile([128, N], F32R, name="yraw", tag="yraw")
                nc.sync.dma_start(out=yraw[yo1:yo1 + 63, :],
                                  in_=kproj[ch][rb + 1:rb + 64, :])
                nc.gpsimd.dma_start(out=yraw[yo1 + 63:yo1 + 64, :],
                                    in_=ns1y[r1:r1 + 1, :])
                nc.sync.dma_start(out=yraw[yo2:yo2 + 62, :],
                                  in_=kproj[ch][rb + 2:rb + 64, :])
                nc.gpsimd.dma_start(out=yraw[yo2 + 62:yo2 + 63, :],
                                    in_=ns1y[r2:r2 + 1, :])
                hole = yo2 + 63  # the single uncovered row
                nc.gpsimd.dma_start(out=yraw[hole:hole + 1, :],
                                    in_=ns1y[r1:r1 + 1, :])

                bb = headp.tile([128, N], F32R, name="bb", tag="bb")
                nc.gpsimd.dma_start(
                    out=bb[yo1:yo1 + 64, :],
                    in_=betad[r1:r1 + 1, :].to_broadcast((64, N)))
                nc.gpsimd.dma_start(
                    out=bb[yo2:yo2 + 64, :],
                    in_=betad[r2:r2 + 1, :].to_broadcast((64, N)))

                yt = headp.tile([128, N], F32R, name="yt", tag="yt")
                nc.gpsimd.tensor_mul(yt, yraw, bb)

                # X2: m2 lhsT block at rows yo2..yo2+62 (q d=0..61 + mx2)
                x2 = headp.tile([128, N], F32R, name="x2", tag="x2")
                nc.sync.dma_start(out=x2[yo2:yo2 + 62, :],
                                  in_=qproj[ch][rb:rb + 62, :])
                nc.gpsimd.dma_start(out=x2[yo2 + 62:yo2 + 63, :],
                                    in_=mx[r2:r2 + 1, :])
                heads[h] = (yt, x2)

            def is_typeb(t):
                # ~80% of iters evacuate G2 on DVE; every 5th uses ACT for
                # both evacs. Balances ACT vs DVE measured per-op costs.
                return (t % 5) != 4

            st = {}

            def emit_pe(t):
                h, ic = divmod(t, 8)
                ch, rb = h // 2, (h % 2) * 64
                yo1, yo2 = rb, 64 - rb
                yt, x2 = heads[h]
                isl = slice(ic * 128, (ic + 1) * 128)
                psg1 = gps.tile([128, N], F32, name="psg1", tag="psg1")
                psg2 = gps.tile([128, N], F32, name="psg2", tag="psg2")
                for jh in range(2):
                    jsl = slice(jh * 512, (jh + 1) * 512)
                    nc.tensor.matmul(psg1[:, jsl],
                                     lhsT=qproj[ch][rb:rb + 64, isl],
                                     rhs=yt[yo1:yo1 + 64, jsl],
                                     start=True, stop=True)
                    nc.tensor.matmul(psg2[:, jsl],
                                     lhsT=x2[yo2:yo2 + 63, isl],
                                     rhs=yt[yo2:yo2 + 63, jsl],
                                     start=True, stop=True)
                st[t] = {"psg1": psg1, "psg2": psg2}

            def emit_evac_act(t):
                h, ic = divmod(t, 8)
                r1, r2 = 2 * h, 2 * h + 1
                s = st[t]
                a1 = alphaT[:, ic * 16 + r1:ic * 16 + r1 + 1]
                a2 = alphaT[:, ic * 16 + r2:ic * 16 + r2 + 1]
                sg1 = sgp.tile([128, N], F16, name="sg1", tag="sg1")
                nc.scalar.activation(out=sg1, in_=s["psg1"], func=AF.Square,
                                     bias=0.0, scale=a1)
                s["sg1"] = sg1
                if not is_typeb(t):
                    sg2 = sgp.tile([128, N], F16, name="sg2", tag="sg2")
                    nc.scalar.activation(out=sg2, in_=s["psg2"],
                                         func=AF.Square, bias=0.0, scale=a2)
                    s["sg2"] = sg2

            def emit_evac_dve(t):
                h, ic = divmod(t, 8)
                r2 = 2 * h + 1
                s = st[t]
                if is_typeb(t):
                    a2 = alphaT[:, ic * 16 + r2:ic * 16 + r2 + 1]
                    e2 = sgp.tile([128, N], F16, name="e2", tag="e2")
                    nc.vector.tensor_scalar(out=e2, in0=s["psg2"], scalar1=a2,
                                            scalar2=None, op0=OP.mult)
                    sq2 = sgp.tile([128, N], F16, name="sq2", tag="sq2")
                    nc.vector.tensor_mul(sq2, e2, e2)
                    s["sg2"] = sq2

            def emit_glue(t):
                s = st[t]
                h1 = uup.tile([128, N], F16, name="h1", tag="h1")
                nc.vector.tensor_scalar(out=h1, in0=s["sg1"], scalar1=-1.0,
                                        scalar2=C, op0=OP.mult, op1=OP.add)
                h2 = uup.tile([128, N], F16, name="h2", tag="h2")
                nc.vector.tensor_scalar(out=h2, in0=s["sg2"], scalar1=-1.0,
                                        scalar2=C, op0=OP.mult, op1=OP.add)
                u = uup.tile([128, N], F16, name="u", tag="u")
                nc.vector.tensor_mul(u, h1, h2)
                s["u"] = u

            def emit_ln(t):
                s = st[t]
                lt = ltp.tile([128, N], F16, name="lt", tag="lt")
                nc.scalar.activation(out=lt, in_=s["u"], func=AF.Ln,
                                     bias=0.0, scale=1.0)
                s["lt"] = lt

            def emit_out(t):
                h, ic = divmod(t, 8)
                isl = slice(ic * 128, (ic + 1) * 128)
                s = st.pop(t)
                o = oop.tile([128, N], F16, name="o", tag="o")
                nc.vector.tensor_scalar(out=o, in0=s["lt"], scalar1=-0.5,
                                        scalar2=None, op0=OP.mult)
                nc.sync.dma_start(out=out[h, isl, :], in_=o)

            prep_head(0)
            prep_head(1)
            for step in range(T + 4):
                if step < T:
                    emit_pe(step)
                    h, ic = divmod(step, 8)
                    if ic == 7 and h + 2 < H:
                        prep_head(h + 2)
                if 0 <= step - 1 < T:
                    emit_evac_act(step - 1)
                    emit_evac_dve(step - 1)
                if 0 <= step - 2 < T:
                    emit_glue(step - 2)
                if 0 <= step - 3 < T:
                    emit_ln(step - 3)
                if 0 <= step - 4 < T:
                    emit_out(step - 4)


def _build_nc(repeat=1):
    nc = bacc.Bacc("TRN2", target_bir_lowering=False)

    qT = nc.dram_tensor("qT", [DF, N], F32, kind="ExternalInput")
    kT = nc.dram_tensor("kT", [DF, N], F32, kind="ExternalInput")
    wqT = nc.dram_tensor("wqT", [DF, 512], F32, kind="ExternalInput")
    wkT = nc.dram_tensor("wkT", [DF, 512], F32, kind="ExternalInput")
    bq = nc.dram_tensor("bq", [128, 4], F32, kind="ExternalInput")
    bk = nc.dram_tensor("bk", [128, 4], F32, kind="ExternalInput")
    xmask = nc.dram_tensor("xmask", [128, 64], F32, kind="ExternalInput")
    ymask = nc.dram_tensor("ymask", [128, 64], F32, kind="ExternalInput")
    invn = nc.dram_tensor("invn", [16, 1], F32, kind="ExternalInput")
    ident = nc.dram_tensor("ident", [128, 128], F32, kind="ExternalInput")
    out = nc.dram_tensor("out", [H, N, N], F16, kind="ExternalOutput")
    # DRAM bounce buffer for beta: SBUF sources cannot use partition-step-0
    # (broadcast) APs, DRAM sources can.
    betad = nc.dram_tensor("betad", [16, N], F32R, kind="Internal")

    t = (qT, kT, wqT, wkT, bq, bk, xmask, ymask, invn, ident, out, betad)
    with tile.TileContext(nc) as tc:
        for _rep in range(repeat):
            _emit_body(nc, tc, t)
    nc.compile()
    return nc


_NC = None


def _get_nc():
    global _NC
    if _NC is None:
        _NC = _build_nc()
    return _NC


def _host_inputs(queries, keys, Wq_w, Wq_b, Wk_w, Wk_b):
    qT = np.ascontiguousarray(queries.transpose(0, 2, 1), dtype=np.float32)
    kT = np.ascontiguousarray(keys.transpose(0, 2, 1), dtype=np.float32)
    wqT = np.ascontiguousarray(Wq_w.T, dtype=np.float32)
    wkT = np.ascontiguousarray(Wk_w.T, dtype=np.float32)
    bq = np.ascontiguousarray(Wq_b.reshape(4, 128).T, dtype=np.float32)
    bk = np.ascontiguousarray(Wk_b.reshape(4, 128).T, dtype=np.float32)

    xmask = np.zeros((128, 64), dtype=np.float32)
    ymask = np.zeros((128, 64), dtype=np.float32)
    for c in range(4):
        for hp in range(2):
            for m in (1, 2):
                j = 4 * c + 2 * hp + (m - 1)      # output partition row r
                col = 16 * c + j                   # column within this chunk's mask
                rows = np.arange(hp * 64, hp * 64 + 64 - m)
                xmask[rows, col] = 1.0
                yrows = np.arange(hp * 64 + m, hp * 64 + 64)
                ymask[yrows, col] = 1.0

    invn = np.array([[1.0 / (64 - ((r % 2) + 1))] for r in range(16)],
                    dtype=np.float32)
    ident = np.eye(128, dtype=np.float32)

    shared = dict(wqT=wqT, wkT=wkT, bq=bq, bk=bk, xmask=xmask, ymask=ymask,
                  invn=invn, ident=ident)
    in_maps = []
    for b in range(B):
        m = dict(shared)
        m["qT"] = np.ascontiguousarray(qT[b])
        m["kT"] = np.ascontiguousarray(kT[b])
        in_maps.append(m)
    return in_maps


def kernel(queries, keys, Wq_w, Wq_b, Wk_w, Wk_b):
    nc = _get_nc()
    in_maps = _host_inputs(np.asarray(queries), np.asarray(keys),
                           np.asarray(Wq_w), np.asarray(Wq_b),
                           np.asarray(Wk_w), np.asarray(Wk_b))
    res = run_bass_kernel_spmd(nc, in_maps, core_ids=list(range(B)))
    out = np.stack([res.results[b]["out"].astype(np.float32) for b in range(B)],
                   axis=0)
    return out


# revision 26
# speedup vs baseline: 25697.3429x; 1.2468x over previous
"""EntropyGraph Trainium2 kernel.

Computes, per batch b (one NeuronCore per batch):
  qt = heads(queries @ Wq_w.T + Wq_b), kt = heads(keys @ Wk_w.T + Wk_b)
  out[b,h,i,j] = -0.5 * sum_m log(1 - corr_m(i,j)^2 + eps)
where corr_m is the lag-m cross-correlation between query series i and key
series j within each head.

Strategy: correlation = alpha_i * G[i,j] where G = PE Gram matmul of
(raw q rows + mean-augmentation row) against (beta-scaled k rows +
-s1y-augmentation row); one-sided centering makes the mean correction exact.
alpha folds into the per-partition scale of the PSUM-evacuation passes.
Elementwise stage: ACT Square -> f16 DVE chain -> single ACT Ln -> DVE
final scale. Plain copies ride HWDGE (nc.sync); only broadcasts use SWDGE.
"""

import sys

import numpy as np

sys.path.insert(0, "/opt/trn_rl_repo")

import concourse.bacc as bacc
import concourse.tile as tile
from concourse import mybir
from concourse.bass_utils import run_bass_kernel_spmd

F32 = mybir.dt.float32
F32R = mybir.dt.float32r
F16 = mybir.dt.float16
OP = mybir.AluOpType
AF = mybir.ActivationFunctionType

B, N, DF = 8, 1024, 128
H, DK = 8, 64
EPS = 1e-6
C = 1.0 + EPS
NCHUNK = 4  # o-chunks of 128 in the 512-wide projection
_ABLATE_OUT_DMA = False  # timing probe only: skip per-iter output DMAs
_MIX_NUM, _MIX_DEN = 1, 2  # type-B (DVE-evac) iteration fraction
_ITER_LIMIT = None  # timing probe only: cap Stage E iterations
_SPLIT_PROJ_EVAC = False  # alternate projection-evac between ACT and DVE


def _emit_body(nc, tc, t):
    qT, kT, wqT, wkT, bq, bk, xmask, ymask, invn, ident, out, betad = t
    with tc.tile_pool(name="const", bufs=1) as const, \
         tc.tile_pool(name="proj", bufs=1) as projp, \
         tc.tile_pool(name="stats", bufs=1) as statp, \
         tc.tile_pool(name="scratch", bufs=2) as scratch:

        # ---- Stage A: load inputs -------------------------------------
        qT_s = const.tile([DF, N], F32)
        kT_s = const.tile([DF, N], F32)
        wqT_s = const.tile([DF, 512], F32)
        wkT_s = const.tile([DF, 512], F32)
        bq_s = const.tile([128, 4], F32)
        bk_s = const.tile([128, 4], F32)
        xm_s = const.tile([128, 64], F32)
        ym_s = const.tile([128, 64], F32)
        invn_s = const.tile([16, 1], F32)
        id_s = const.tile([128, 128], F32)
        for dst, src in ((qT_s, qT), (kT_s, kT), (wqT_s, wqT),
                         (wkT_s, wkT), (bq_s, bq), (bk_s, bk),
                         (xm_s, xmask), (ym_s, ymask), (invn_s, invn),
                         (id_s, ident)):
            nc.sync.dma_start(out=dst, in_=src[:, :])

        # masks rounded to f32r for the (f32r) stats matmuls
        xm_r = const.tile([128, 64], F32R)
        ym_r = const.tile([128, 64], F32R)
        nc.scalar.copy(xm_r, xm_s)
        nc.scalar.copy(ym_r, ym_s)

        # ---- Stage B: projections (transposed layout) -----------------
        # projT[o, n] = W[o, :] @ inT[:, n] + b[o]; fp32 matmul (inputs
        # come straight from DMA so they cannot be f32r), ACT evac
        # rounds to f32r and adds the per-partition bias.
        qproj = []
        kproj = []
        with tc.tile_pool(name="pps", bufs=2, space="PSUM") as pps:
            for (src_s, w_s, b_s, dst_list) in (
                    (kT_s, wkT_s, bk_s, kproj),
                    (qT_s, wqT_s, bq_s, qproj)):
                for c in range(NCHUNK):
                    psb = pps.tile([128, N], F32)
                    for jh in range(2):
                        nc.tensor.matmul(
                            psb[:, jh * 512:(jh + 1) * 512],
                            lhsT=w_s[:, c * 128:(c + 1) * 128],
                            rhs=src_s[:, jh * 512:(jh + 1) * 512],
                            start=True, stop=True)
                    pt = projp.tile([128, N], F32R, tag=f"proj{len(dst_list)}_{c}_{w_s is wkT_s}")
                    # optionally alternate evac engines so the prologue is
                    # not serialized on ACT
                    if _SPLIT_PROJ_EVAC and c % 2 == 1:
                        nc.vector.tensor_scalar(out=pt, in0=psb, scalar1=1.0,
                                                scalar2=b_s[:, c:c + 1],
                                                op0=OP.mult, op1=OP.add)
                    else:
                        nc.scalar.activation(out=pt, in_=psb, func=AF.Identity,
                                             bias=b_s[:, c:c + 1], scale=1.0)
                    dst_list.append(pt)

        # ---- Stage C: raw moments via mask matmuls --------------------
        # s1[r, n] = sum_d proj[d, n] over the (head, m) range; r = 2h+m-1
        # s2 likewise over squared projections.
        sq_list = {}
        with tc.tile_pool(name="sqp", bufs=2) as sqp:
            for name, plist in (("k", kproj), ("q", qproj)):
                for c in range(NCHUNK):
                    sq = sqp.tile([128, N], F32R, tag=f"sq{name}{c}")
                    nc.scalar.activation(out=sq, in_=plist[c],
                                         func=AF.Square, bias=0.0, scale=1.0)
                    sq_list[(name, c)] = sq

            stats_sb = {}
            with tc.tile_pool(name="sps", bufs=1, space="PSUM") as sps:
                for name, plist, mask in (("k", kproj, ym_r),
                                          ("q", qproj, xm_r)):
                    ps1 = sps.tile([16, N], F32, tag=f"ps1{name}")
                    ps2 = sps.tile([16, N], F32, tag=f"ps2{name}")
                    for c in range(NCHUNK):
                        for jh in range(2):
                            sl = slice(jh * 512, (jh + 1) * 512)
                            nc.tensor.matmul(
                                ps1[:, sl],
                                lhsT=mask[:, 16 * c:16 * c + 16],
                                rhs=plist[c][:, sl],
                                start=(c == 0), stop=(c == NCHUNK - 1))
                            nc.tensor.matmul(
                                ps2[:, sl],
                                lhsT=mask[:, 16 * c:16 * c + 16],
                                rhs=sq_list[(name, c)][:, sl],
                                start=(c == 0), stop=(c == NCHUNK - 1))
                    s1 = statp.tile([16, N], F32, tag=f"s1{name}")
                    s2 = statp.tile([16, N], F32, tag=f"s2{name}")
                    nc.scalar.copy(s1, ps1)
                    nc.scalar.copy(s2, ps2)
                    stats_sb[name] = (s1, s2)

        # ---- Stage D: stats math --------------------------------------
        s1q, s2q = stats_sb["q"]
        s1k, s2k = stats_sb["k"]
        invn_ap = invn_s[:, 0:1]

        # k-side: ns1y = -s1y (f32r, aug rows); beta = 1/sqrt(ssy) (f32r)
        ns1y = statp.tile([16, N], F32R)
        nc.vector.tensor_scalar(out=ns1y, in0=s1k, scalar1=-1.0,
                                scalar2=None, op0=OP.mult)
        tk = scratch.tile([16, N], F32, tag="tq")
        nc.vector.tensor_mul(tk, s1k, s1k)
        nssy = scratch.tile([16, N], F32, tag="nssx")
        nc.vector.scalar_tensor_tensor(out=nssy, in0=tk, scalar=invn_ap,
                                       in1=s2k, op0=OP.mult, op1=OP.subtract)
        ryn = scratch.tile([16, N], F32, tag="rxn")
        nc.vector.reciprocal(ryn, nssy)
        beta16 = statp.tile([16, N], F32R)
        nc.scalar.activation(out=beta16, in_=ryn, func=AF.Sqrt,
                             bias=0.0, scale=-1.0)
        nc.sync.dma_start(out=betad[:, :], in_=beta16)

        # mx = s1x / n_eff  (f32r: feeds augmentation rows via DMA)
        mx = statp.tile([16, N], F32R)
        nc.vector.tensor_scalar(out=mx, in0=s1q, scalar1=invn_ap,
                                scalar2=None, op0=OP.mult)
        # -ssx = s1x^2/n - s2x ; alpha = sqrt(-1/(-ssx)) = 1/sqrt(ssx)
        tq = scratch.tile([16, N], F32, tag="tq")
        nc.vector.tensor_mul(tq, s1q, s1q)
        nssx = scratch.tile([16, N], F32, tag="nssx")
        nc.vector.scalar_tensor_tensor(out=nssx, in0=tq, scalar=invn_ap,
                                       in1=s2q, op0=OP.mult, op1=OP.subtract)
        rxn = scratch.tile([16, N], F32, tag="rxn")
        nc.vector.reciprocal(rxn, nssx)
        alpha16 = statp.tile([16, N], F32)
        nc.scalar.activation(out=alpha16, in_=rxn, func=AF.Sqrt,
                             bias=0.0, scale=-1.0)

        # alpha transposed to [128, 8*16]: col ic*16 + (2h + m - 1)
        alphaT = statp.tile([128, 128], F32)
        with tc.tile_pool(name="tps", bufs=1, space="PSUM") as tps:
            pst = tps.tile([128, 128], F32)
            for ic in range(8):
                nc.tensor.transpose(pst[:, ic * 16:(ic + 1) * 16],
                                    in_=alpha16[:, ic * 128:(ic + 1) * 128],
                                    identity=id_s[0:16, 0:16])
            nc.scalar.copy(alphaT, pst)

        # m1 augmentation: overwrite q_projT row rb+63 (unused d=63) with mx1
        for h in range(H):
            ch, rb = h // 2, (h % 2) * 64
            nc.sync.dma_start(out=qproj[ch][rb + 63:rb + 64, :],
                                in_=mx[2 * h:2 * h + 1, :])

        # ---- Stage E: per-head Grams + elementwise (software-pipelined)
        # Flat iteration t = 8*h + ic.  Stages at skews:
        #   step t+0: PE Gram matmuls -> psg1/psg2
        #   step t+1: PSUM evacuation (ACT Square; type-B iters put the
        #             G2 evac on DVE instead to balance engine load)
        #   step t+2: DVE glue -> u = (C - s1)(C - s2)
        #   step t+3: ACT Ln
        #   step t+4: DVE -0.5 scale + DMA out
        # Emission is stage-sorted so neither engine's FIFO head blocks on
        # the other engine's in-flight work.
        T = H * 8
        with tc.tile_pool(name="head", bufs=2) as headp, \
             tc.tile_pool(name="sgp", bufs=3) as sgp, \
             tc.tile_pool(name="uup", bufs=2) as uup, \
             tc.tile_pool(name="uwp", bufs=2) as uwp, \
             tc.tile_pool(name="ltp", bufs=2) as ltp, \
             tc.tile_pool(name="oop", bufs=3) as oop, \
             tc.tile_pool(name="gps", bufs=2, space="PSUM") as gps:

            heads = {}

            def prep_head(h):
                ch, rb = h // 2, (h % 2) * 64
                yo1, yo2 = rb, 64 - rb
                r1, r2 = 2 * h, 2 * h + 1
                # Y raw: m1 block rows yo1..yo1+63 (k d=1..63 + aug),
                #        m2 block rows yo2..yo2+62 (k d=2..63 + aug)
                yraw = headp.tile([128, N], F32R, name="yraw", tag="yraw")
                nc.sync.dma_start(out=yraw[yo1:yo1 + 63, :],
                                  in_=kproj[ch][rb + 1:rb + 64, :])
                nc.gpsimd.dma_start(out=yraw[yo1 + 63:yo1 + 64, :],
                                    in_=ns1y[r1:r1 + 1, :])
                nc.sync.dma_start(out=yraw[yo2:yo2 + 62, :],
                                  in_=kproj[ch][rb + 2:rb + 64, :])
                nc.gpsimd.dma_start(out=yraw[yo2 + 62:yo2 + 63, :],
                                    in_=ns1y[r2:r2 + 1, :])
                hole = yo2 + 63  # the single uncovered row
                nc.gpsimd.dma_start(out=yraw[hole:hole + 1, :],
                                    in_=ns1y[r1:r1 + 1, :])

                bb = headp.tile([128, N], F32R, name="bb", tag="bb")
                nc.gpsimd.dma_start(
                    out=bb[yo1:yo1 + 64, :],
                    in_=betad[r1:r1 + 1, :].to_broadcast((64, N)))
                nc.gpsimd.dma_start(
                    out=bb[yo2:yo2 + 64, :],
                    in_=betad[r2:r2 + 1, :].to_broadcast((64, N)))

                yt = headp.tile([128, N], F32R, name="yt", tag="yt")
                nc.gpsimd.tensor_mul(yt, yraw, bb)

                # X2: m2 lhsT block at rows yo2..yo2+62 (q d=0..61 + mx2)
                x2 = headp.tile([128, N], F32R, name="x2", tag="x2")
                nc.sync.dma_start(out=x2[yo2:yo2 + 62, :],
                                  in_=qproj[ch][rb:rb + 62, :])
                nc.gpsimd.dma_start(out=x2[yo2 + 62:yo2 + 63, :],
                                    in_=mx[r2:r2 + 1, :])
                heads[h] = (yt, x2)

            def is_typeb(t):
                # Fraction of iters that evacuate G2 on DVE (the rest use a
                # second ACT Square). Balances ACT vs DVE load.
                return (t % _MIX_DEN) < _MIX_NUM

            st = {}

            def emit_pe(t):
                h, ic = divmod(t, 8)
                ch, rb = h // 2, (h % 2) * 64
                yo1, yo2 = rb, 64 - rb
                yt, x2 = heads[h]
                isl = slice(ic * 128, (ic + 1) * 128)
                psg1 = gps.tile([128, N], F32, name="psg1", tag="psg1")
                psg2 = gps.tile([128, N], F32, name="psg2", tag="psg2")
                for jh in range(2):
                    jsl = slice(jh * 512, (jh + 1) * 512)
                    nc.tensor.matmul(psg1[:, jsl],
                                     lhsT=qproj[ch][rb:rb + 64, isl],
                                     rhs=yt[yo1:yo1 + 64, jsl],
                                     start=True, stop=True)
                    nc.tensor.matmul(psg2[:, jsl],
                                     lhsT=x2[yo2:yo2 + 63, isl],
                                     rhs=yt[yo2:yo2 + 63, jsl],
                                     start=True, stop=True)
                st[t] = {"psg1": psg1, "psg2": psg2}

            def emit_evac_act(t):
                h, ic = divmod(t, 8)
                r1, r2 = 2 * h, 2 * h + 1
                s = st[t]
                a1 = alphaT[:, ic * 16 + r1:ic * 16 + r1 + 1]
                a2 = alphaT[:, ic * 16 + r2:ic * 16 + r2 + 1]
                sg1 = sgp.tile([128, N], F16, name="sg1", tag="sg1")
                nc.scalar.activation(out=sg1, in_=s["psg1"], func=AF.Square,
                                     bias=0.0, scale=a1)
                s["sg1"] = sg1
                if not is_typeb(t):
                    sg2 = sgp.tile([128, N], F16, name="sg2", tag="sg2")
                    nc.scalar.activation(out=sg2, in_=s["psg2"],
                                         func=AF.Square, bias=0.0, scale=a2)
                    s["sg2"] = sg2

            def emit_evac_dve(t):
                h, ic = divmod(t, 8)
                r2 = 2 * h + 1
                s = st[t]
                if is_typeb(t):
                    a2 = alphaT[:, ic * 16 + r2:ic * 16 + r2 + 1]
                    e2 = sgp.tile([128, N], F16, name="e2", tag="e2")
                    nc.vector.tensor_scalar(out=e2, in0=s["psg2"], scalar1=a2,
                                            scalar2=None, op0=OP.mult)
                    sq2 = sgp.tile([128, N], F16, name="sq2", tag="sq2")
                    nc.vector.tensor_mul(sq2, e2, e2)
                    s["sg2"] = sq2

            pairs = {}

            def emit_glue(t):
                s = st.pop(t)
                p, half = divmod(t, 2)
                if half == 0:
                    pairs[p] = {
                        "h1w": uup.tile([128, 2 * N], F16, name="h1w",
                                        tag="h1w"),
                        "h2w": uup.tile([128, 2 * N], F16, name="h2w",
                                        tag="h2w"),
                    }
                pw = pairs[p]
                sl = slice(half * N, (half + 1) * N)
                nc.vector.tensor_scalar(out=pw["h1w"][:, sl], in0=s["sg1"],
                                        scalar1=-1.0, scalar2=C,
                                        op0=OP.mult, op1=OP.add)
                nc.vector.tensor_scalar(out=pw["h2w"][:, sl], in0=s["sg2"],
                                        scalar1=-1.0, scalar2=C,
                                        op0=OP.mult, op1=OP.add)

            def emit_upair(p):
                pw = pairs[p]
                u = uwp.tile([128, 2 * N], F16, name="uw", tag="uw")
                nc.vector.tensor_mul(u, pw["h1w"], pw["h2w"])
                pw["u"] = u

            def emit_lnpair(p):
                pw = pairs[p]
                ltw = ltp.tile([128, 2 * N], F16, name="ltw", tag="ltw")
                nc.scalar.activation(out=ltw, in_=pw["u"], func=AF.Ln,
                                     bias=0.0, scale=1.0)
                pw["lt"] = ltw

            def emit_tailpair(p):
                pw = pairs.pop(p)
                ow = oop.tile([128, 2 * N], F16, name="ow", tag="ow")
                nc.vector.tensor_scalar(out=ow, in0=pw["lt"], scalar1=-0.5,
                                        scalar2=None, op0=OP.mult)
                for half in range(2):
                    t = 2 * p + half
                    if _ABLATE_OUT_DMA and t != T - 1:
                        continue
                    h, ic = divmod(t, 8)
                    isl = slice(ic * 128, (ic + 1) * 128)
                    # alternate HWDGE (SP ring) and SWDGE (pool ring) so
                    # the output stores do not serialize on one DMA FIFO
                    eng = nc.sync if half == 0 else nc.gpsimd
                    eng.dma_start(out=out[h, isl, :],
                                  in_=ow[:, half * N:(half + 1) * N])

            Tl = _ITER_LIMIT if _ITER_LIMIT is not None else T
            assert Tl % 2 == 0
            hmax = (Tl - 1) // 8
            prep_head(0)
            if hmax >= 1:
                prep_head(1)
            for step in range(Tl + 7):
                if step < Tl:
                    emit_pe(step)
                    h, ic = divmod(step, 8)
                    if ic == 7 and h + 2 <= hmax:
                        prep_head(h + 2)
                if 0 <= step - 1 < Tl:
                    emit_evac_act(step - 1)
                    emit_evac_dve(step - 1)
                if 0 <= step - 2 < Tl:
                    emit_glue(step - 2)
                for off, fn in ((4, emit_upair), (5, emit_lnpair),
                                (6, emit_tailpair)):
                    q = step - off
                    if q >= 0 and q % 2 == 0 and q < Tl:
                        fn(q // 2)


def _build_nc(repeat=1):
    nc = bacc.Bacc("TRN2", target_bir_lowering=False)

    qT = nc.dram_tensor("qT", [DF, N], F32, kind="ExternalInput")
    kT = nc.dram_tensor("kT", [DF, N], F32, kind="ExternalInput")
    wqT = nc.dram_tensor("wqT", [DF, 512], F32, kind="ExternalInput")
    wkT = nc.dram_tensor("wkT", [DF, 512], F32, kind="ExternalInput")
    bq = nc.dram_tensor("bq", [128, 4], F32, kind="ExternalInput")
    bk = nc.dram_tensor("bk", [128, 4], F32, kind="ExternalInput")
    xmask = nc.dram_tensor("xmask", [128, 64], F32, kind="ExternalInput")
    ymask = nc.dram_tensor("ymask", [128, 64], F32, kind="ExternalInput")
    invn = nc.dram_tensor("invn", [16, 1], F32, kind="ExternalInput")
    ident = nc.dram_tensor("ident", [128, 128], F32, kind="ExternalInput")
    out = nc.dram_tensor("out", [H, N, N], F16, kind="ExternalOutput")
    # DRAM bounce buffer for beta: SBUF sources cannot use partition-step-0
    # (broadcast) APs, DRAM sources can.
    betad = nc.dram_tensor("betad", [16, N], F32R, kind="Internal")

    t = (qT, kT, wqT, wkT, bq, bk, xmask, ymask, invn, ident, out, betad)
    with tile.TileContext(nc) as tc:
        for _rep in range(repeat):
            _emit_body(nc, tc, t)
    nc.compile()
    return nc


_NC = None


def _get_nc():
    global _NC
    if _NC is None:
        _NC = _build_nc()
    return _NC


def _host_inputs(queries, keys, Wq_w, Wq_b, Wk_w, Wk_b):
    qT = np.ascontiguousarray(queries.transpose(0, 2, 1), dtype=np.float32)
    kT = np.ascontiguousarray(keys.transpose(0, 2, 1), dtype=np.float32)
    wqT = np.ascontiguousarray(Wq_w.T, dtype=np.float32)
    wkT = np.ascontiguousarray(Wk_w.T, dtype=np.float32)
    bq = np.ascontiguousarray(Wq_b.reshape(4, 128).T, dtype=np.float32)
    bk = np.ascontiguousarray(Wk_b.reshape(4, 128).T, dtype=np.float32)

    xmask = np.zeros((128, 64), dtype=np.float32)
    ymask = np.zeros((128, 64), dtype=np.float32)
    for c in range(4):
        for hp in range(2):
            for m in (1, 2):
                j = 4 * c + 2 * hp + (m - 1)      # output partition row r
                col = 16 * c + j                   # column within this chunk's mask
                rows = np.arange(hp * 64, hp * 64 + 64 - m)
                xmask[rows, col] = 1.0
                yrows = np.arange(hp * 64 + m, hp * 64 + 64)
                ymask[yrows, col] = 1.0

    invn = np.array([[1.0 / (64 - ((r % 2) + 1))] for r in range(16)],
                    dtype=np.float32)
    ident = np.eye(128, dtype=np.float32)

    shared = dict(wqT=wqT, wkT=wkT, bq=bq, bk=bk, xmask=xmask, ymask=ymask,
                  invn=invn, ident=ident)
    in_maps = []
    for b in range(B):
        m = dict(shared)
        m["qT"] = np.ascontiguousarray(qT[b])
        m["kT"] = np.ascontiguousarray(kT[b])
        in_maps.append(m)
    return in_maps


def kernel(queries, keys, Wq_w, Wq_b, Wk_w, Wk_b):
    nc = _get_nc()
    in_maps = _host_inputs(np.asarray(queries), np.asarray(keys),
                           np.asarray(Wq_w), np.asarray(Wq_b),
                           np.asarray(Wk_w), np.asarray(Wk_b))
    res = run_bass_kernel_spmd(nc, in_maps, core_ids=list(range(B)))
    out = np.stack([res.results[b]["out"].astype(np.float32) for b in range(B)],
                   axis=0)
    return out
